# revision 48
# baseline (speedup 1.0000x reference)
"""GNN attention layer (nn_Attention_Layer_21131239096479) on 8 TRN2 NeuronCores.

Strategy:
 - LayerNorm+projection algebraically decomposed so the E x 576 x 256 projections
   become per-node tables (x @ B) + per-edge 64-wide matmuls + gathers:
     key[e]   = inv_sig[e]*(ea[e]@A_k + XBk[src] + XCk[dst]) - mu[e]*inv_sig[e]*s_k + c_k
   LayerNorm scalar stats (mu, inv_sigma) per edge are precomputed host-side
   (O(N*C + E) work) and streamed as resident per-edge tables, so the device
   does no per-edge stats math.  Softmax denominators are applied post-scatter,
   so one pass over edges suffices.
 - Edges sharded across 8 cores by dst range (1250 nodes/core); inside a core,
   edges are grouped into 10 windows of 128 dst nodes.  Segment softmax and
   scatter-sum are done with onehot matmuls on the TensorEngine.
 - Src-side table rows (XBk|XBv, 512 bf16, built on-device) are fetched with
   dma_gather, 768 edges per gather.  All matmuls run bf16; x is shipped
   host-pre-transposed so no on-device transposes are needed outside the MLP.
 - Both onehot matrices (edge->dst scatter and its transpose) are built in one
   batched DVE compare each per window, using a host-replicated drel stream.
"""
import math
import numpy as np
from contextlib import ExitStack

import concourse.bass as bass
import concourse.bacc as bacc
import concourse.mybir as mybir
import concourse.tile as tile
import concourse.bass_utils as bass_utils
from concourse import library_config
import ml_dtypes

FP32 = mybir.dt.float32
FP32R = mybir.dt.float32r
BF16 = mybir.dt.bfloat16
I32 = mybir.dt.int32
I16 = mybir.dt.int16
AF = mybir.ActivationFunctionType
ALU = mybir.AluOpType
AX = mybir.AxisListType

N, E = 10000, 320000
CZ, CE, CO, H, CF = 256, 64, 32, 8, 576
NCORES, NLOC, NLOC_PAD, NWIN = 8, 1250, 1280, 10
SBE = 768                      # edges per superblock (j-blocks of 128)
JPS = SBE // 128               # 6
NPAD = 10112                   # padded node-table rows (79*128)
NTB = NPAD // 128              # 79
TROW = 512                     # bf16 table row: XBk(256) XBv(256)


class Cfg:
    def __init__(self, **kw):
        self.__dict__.update(kw)


def _host_prep(cfg, x, edge_index, edge_attr, ln_gamma, ln_beta, Wq, bq, Wk, bk, Wv, bv,
               W1, b1, W2, b2):
    NCORES, NLOC, NLOC_PAD, NWIN, NPAD = (cfg.NCORES, cfg.NLOC, cfg.NLOC_PAD,
                                          cfg.NWIN, cfg.NPAD)
    f32 = np.float32
    x = np.asarray(x, f32)
    ei = np.asarray(edge_index)
    ea = np.asarray(edge_attr, f32)
    gamma = np.asarray(ln_gamma, f32); beta = np.asarray(ln_beta, f32)
    Wq = np.asarray(Wq, f32) / math.sqrt(CO); bq = np.asarray(bq, f32) / math.sqrt(CO)
    Wk = np.asarray(Wk, f32); bk = np.asarray(bk, f32)
    Wv = np.asarray(Wv, f32); bv = np.asarray(bv, f32)
    W1 = np.asarray(W1, f32); b1 = np.asarray(b1, f32)
    W2 = np.asarray(W2, f32); b2 = np.asarray(b2, f32)

    Wkg = Wk * gamma[:, None]; Wvg = Wv * gamma[:, None]
    A_k, B_k, C_k = Wkg[:CE], Wkg[CE:CE + CZ], Wkg[CE + CZ:]
    A_v, B_v, C_v = Wvg[:CE], Wvg[CE:CE + CZ], Wvg[CE + CZ:]
    s_k = Wkg.sum(0); c_k = beta @ Wk + bk
    s_v = Wvg.sum(0); c_v = beta @ Wv + bv

    def chunk_pack(M, kchunks):
        # [K, N] -> [128, kchunks, N] with M[k*128+p, n] at [p, k, n]
        K, Nc = M.shape
        assert K == kchunks * 128
        return np.ascontiguousarray(M.reshape(kchunks, 128, Nc).transpose(1, 0, 2))

    bf16 = ml_dtypes.bfloat16
    consts = {
        "akv": np.concatenate([A_k, A_v], 1).astype(bf16),                 # [64, 512]
        "wqck": chunk_pack(np.concatenate([Wq, C_k], 1), 2).astype(bf16),  # [128,2,512]
        "cv": chunk_pack(C_v, 2).astype(bf16),                             # [128,2,256]
        "bkv": chunk_pack(np.concatenate([B_k, B_v], 1), 2).astype(bf16),  # [128,2,512]
        "w1": chunk_pack(W1, 2).astype(bf16),                              # [128,2,512]
        "w2": chunk_pack(W2, 4).astype(bf16),                              # [128,4,256]
        "skb": np.tile(s_k, (128, 1)).astype(f32),
        "ckb": np.tile(c_k, (128, 1)).astype(f32),
        "svb": np.tile(s_v, (128, 1)).astype(f32),
        "cvb": np.tile(c_v, (128, 1)).astype(f32),
        "bqb": np.tile(bq, (128, 1)).astype(f32),
        "b2b": np.tile(b2, (128, 1)).astype(f32),
        "b1b": np.tile(b1, (128, 1)).astype(f32),
        "identb": np.eye(128).astype(bf16),
        "iota": np.tile(np.arange(128), (128, 1)).astype(bf16),
        "iotap": np.arange(128).reshape(128, 1).astype(f32),
    }

    x_pad = np.zeros((NPAD, CZ), f32); x_pad[:x.shape[0]] = x
    NTB_ = NPAD // 128

    src, dst = ei[0].astype(np.int64), ei[1].astype(np.int64)
    core_of = dst // NLOC

    # host-side LayerNorm scalar stats per edge (O(N*C + E))
    sx = x.sum(1); sqx = (x * x).sum(1)                    # [N]
    se = ea.sum(1); sqe = (ea * ea).sum(1)                 # [E]
    S = se + sx[src] + sx[dst]
    Q = sqe + sqx[src] + sqx[dst]
    mu_e = S / CF
    var_e = Q / CF - mu_e * mu_e
    inv_e = 1.0 / np.sqrt(var_e + 1e-5)
    muinv_e = mu_e * inv_e

    # per-core, per-window grouping
    per_core = []
    maxcnt = 0
    for c in range(NCORES):
        m = core_of == c
        esrc = src[m]; edst = dst[m] - c * NLOC; eat = ea[m]
        einv = inv_e[m]; emuinv = muinv_e[m]
        order = np.argsort(edst, kind="stable")
        esrc, edst, eat = esrc[order], edst[order], eat[order]
        einv, emuinv = einv[order], emuinv[order]
        w = edst // 128
        counts = np.bincount(w, minlength=NWIN)
        maxcnt = max(maxcnt, int(counts.max()))
        per_core.append((esrc, edst, eat, einv, emuinv, counts))

    W_E = int(math.ceil(maxcnt / SBE) * SBE)
    NSB = W_E // SBE
    NSBT = NWIN * NSB

    in_maps = []
    IPS = SBE // 16              # idx cols per superblock (96)
    for c in range(NCORES):
        esrc, edst, eat, einv, emuinv, counts = per_core[c]
        idx16 = np.zeros(NWIN * W_E, np.int16)
        drel = np.full(NWIN * W_E, -1, np.int32)
        invf = np.ones(NWIN * W_E, f32)
        muinvf = np.zeros(NWIN * W_E, f32)
        ea_t = np.zeros((CE, NWIN * W_E), f32)
        pos = 0
        for wi in range(NWIN):
            cnt = int(counts[wi])
            s = wi * W_E
            idx16[s:s + cnt] = esrc[pos:pos + cnt]
            drel[s:s + cnt] = edst[pos:pos + cnt] - wi * 128
            invf[s:s + cnt] = einv[pos:pos + cnt]
            muinvf[s:s + cnt] = emuinv[pos:pos + cnt]
            ea_t[:, s:s + cnt] = eat[pos:pos + cnt].T
            pos += cnt
        # gather index layout: idx j -> [j % 16, j // 16] per superblock, replicated
        IDX = np.zeros((128, NSBT * IPS), np.int16)
        blk = idx16.reshape(NSBT, IPS, 16)           # [sb, s, p]
        for sb in range(NSBT):
            IDX[:16, sb * IPS:(sb + 1) * IPS] = blk[sb].T
        IDX[16:] = np.tile(IDX[:16], (7, 1))

        def col_pack(v):
            # [NWIN*W_E] -> [128, NWIN*W_E//128] per-128-edge column layout
            return np.ascontiguousarray(v.reshape(NWIN * W_E // 128, 128).T)

        DREL = col_pack(drel.astype(np.float32)).astype(ml_dtypes.bfloat16)
        INV = col_pack(invf)
        MUINV = col_pack(muinvf)
        # drel replicated across partitions: [128, NWIN*W_E] bf16 for OHd build
        DRELR = np.broadcast_to(drel.astype(np.float32).astype(ml_dtypes.bfloat16)[None, :],
                                (128, NWIN * W_E)).copy()
        x_loc = np.zeros((NLOC_PAD, CZ), f32)
        x_loc[:NLOC] = x[c * NLOC:(c + 1) * NLOC]
        # host-pre-transposed x tables (bf16):
        #  XT3[p, b, k, n] = x_pad[b*128+n, k*128+p];  XLT[p, w, k, n] similarly
        XT3 = np.ascontiguousarray(
            x_pad.reshape(NTB_, 128, 2, 128).transpose(3, 0, 2, 1)).astype(ml_dtypes.bfloat16)
        XLT = np.ascontiguousarray(
            x_loc.reshape(NWIN, 128, 2, 128).transpose(3, 0, 2, 1)).astype(ml_dtypes.bfloat16)
        in_maps.append({
            "xt3": XT3, "xlt": XLT,
            "ea_t": np.ascontiguousarray(ea_t).astype(ml_dtypes.bfloat16),
            "idx": IDX, "drel": DREL, "drelr": DRELR, "inv": INV, "muinv": MUINV,
        })
    return consts, in_maps, W_E, NSB


def _build(nc, tc, ctx, consts_h, cfg, ins=None, outs=None):
    """Emit the kernel IR.  If ins/outs given (sim path), use those APs."""
    NLOC_PAD, NWIN, NPAD, NTB, W_E, NSB = (cfg.NLOC_PAD, cfg.NWIN, cfg.NPAD,
                                           cfg.NTB, cfg.W_E, cfg.NSB)
    stage = getattr(cfg, "stage", 99)
    JPW = W_E // 128             # j-blocks per window (36)
    IPS = SBE // 16              # idx cols per superblock (96)
    if ins is None:
        xt3_d = nc.dram_tensor("xt3", [128, NTB, 2, 128], BF16, kind="ExternalInput").ap()
        xlt_d = nc.dram_tensor("xlt", [128, NWIN, 2, 128], BF16, kind="ExternalInput").ap()
        ea_t = nc.dram_tensor("ea_t", [CE, NWIN * W_E], BF16, kind="ExternalInput").ap()
        idx_d = nc.dram_tensor("idx", [128, NWIN * NSB * IPS], I16, kind="ExternalInput").ap()
        drel_d = nc.dram_tensor("drel", [128, NWIN * JPW], BF16, kind="ExternalInput").ap()
        drelr_d = nc.dram_tensor("drelr", [128, NWIN * W_E], BF16, kind="ExternalInput").ap()
        inv_d = nc.dram_tensor("inv", [128, NWIN * JPW], FP32, kind="ExternalInput").ap()
        muinv_d = nc.dram_tensor("muinv", [128, NWIN * JPW], FP32, kind="ExternalInput").ap()
        y_d = nc.dram_tensor("y", [NLOC_PAD, CZ], FP32, kind="ExternalOutput").ap()
    else:
        xt3_d, xlt_d, ea_t, idx_d, drel_d = (ins["xt3"], ins["xlt"], ins["ea_t"],
                                             ins["idx"], ins["drel"])
        drelr_d = ins["drelr"]
        inv_d, muinv_d = ins["inv"], ins["muinv"]
        y_d = outs["y"]
    tsrc = nc.dram_tensor("tsrc", [NPAD, TROW], BF16, kind="Internal").ap()

    cd = {k: nc.inline_tensor(np.asarray(v), name=f"c_{k}").ap() for k, v in consts_h.items()}
    nc.gpsimd.load_library(library_config.mlp)

    # ---------------- resident constants in SBUF ----------------
    cpool = ctx.enter_context(tc.tile_pool(name="consts", bufs=1))
    cs = {}
    for k, ap in cd.items():
        t = cpool.tile(list(ap.shape), ap.dtype, tag=f"c_{k}")
        nc.sync.dma_start(t[:], ap)
        cs[k] = t
    # resident per-core index data
    idx_sb = cpool.tile([128, NWIN * NSB * IPS], I16, tag="idxsb")
    nc.sync.dma_start(idx_sb[:], idx_d)
    drel_sb = cpool.tile([128, NWIN * JPW], BF16, tag="drelsb")
    nc.sync.dma_start(drel_sb[:], drel_d)
    inv_sb = cpool.tile([128, NWIN * JPW], FP32, tag="invsb")
    nc.sync.dma_start(inv_sb[:], inv_d)
    muinv_sb = cpool.tile([128, NWIN * JPW], FP32, tag="muinvsb")
    nc.sync.dma_start(muinv_sb[:], muinv_d)

    # ---------------- pools ----------------
    # PSUM (8 banks): p_kv 3 + p_q 2 + p_scat 1 + p_tpb 2 = 8
    p_kv = ctx.enter_context(tc.tile_pool(name="p_kv", bufs=3, space="PSUM"))     # [128,512] f32
    p_q = ctx.enter_context(tc.tile_pool(name="p_q", bufs=3, space="PSUM"))       # [128,280] f32
    p_scat = ctx.enter_context(tc.tile_pool(name="p_scat", bufs=1, space="PSUM"))  # [128,280] f32
    p_tpb = ctx.enter_context(tc.tile_pool(name="p_tpb", bufs=1, space="PSUM"))   # [128,4,128] bf16

    sb_tab = ctx.enter_context(tc.tile_pool(name="sb_tab", bufs=3))
    sb_ea = ctx.enter_context(tc.tile_pool(name="sb_ea", bufs=2))
    sb_dr = ctx.enter_context(tc.tile_pool(name="sb_dr", bufs=1))
    sb_kvv = ctx.enter_context(tc.tile_pool(name="sb_kvv", bufs=1))
    sb_od = ctx.enter_context(tc.tile_pool(name="sb_od", bufs=1))
    sb_msg = ctx.enter_context(tc.tile_pool(name="sb_msg", bufs=3))
    sb_gt = ctx.enter_context(tc.tile_pool(name="sb_gt", bufs=3))
    sb_oh = ctx.enter_context(tc.tile_pool(name="sb_oh", bufs=2))
    sb_work = ctx.enter_context(tc.tile_pool(name="sb_work", bufs=2))
    sb_win = ctx.enter_context(tc.tile_pool(name="sb_win", bufs=2))

    def transpose_128(out_ps, in_sb):
        nc.tensor.transpose(out_ps, in_sb, cs["identb"][:])

    # ================= phase A: build src table =================
    for b in range(NTB):
        xt = sb_tab.tile([128, 2, 128], BF16, tag="xt")
        nc.sync.dma_start(xt[:], xt3_d[:, b, :, :])
        mm = p_kv.tile([128, 512], FP32, tag="kvp")
        for k in range(2):
            nc.tensor.matmul(mm[:], xt[:, k, :],
                             cs["bkv"][:, k, :],
                             start=(k == 0), stop=(k == 1))
        to = sb_tab.tile([128, TROW], BF16, tag="to")
        nc.vector.tensor_copy(to[:], mm[:])
        nc.scalar.dma_start(tsrc[b * 128:(b + 1) * 128, :], to[:])

    if stage < 2:
        dbg = sb_tab.tile([128, CZ], FP32, tag="dbg")
        nc.scalar.copy(dbg[:], to[:, 0:256])
        nc.sync.dma_start(y_d[0:128, :], dbg[:])
        return

    # ================= phase B: windows =================
    for w in range(NWIN):
        # ---- window prep ----
        xwt = sb_win.tile([128, 2, 128], BF16, tag="xwt")
        nc.sync.dma_start(xwt[:], xlt_d[:, w, :, :])

        qx = p_kv.tile([128, 512], FP32, tag="kvp")
        for k in range(2):
            nc.tensor.matmul(qx[:], xwt[:, k, :],
                             cs["wqck"][:, k, :],
                             start=(k == 0), stop=(k == 1))
        qf = sb_win.tile([128, CZ], FP32, tag="qf")
        nc.vector.tensor_add(qf[:], qx[:, 0:256], cs["bqb"][:])
        dcol = sb_win.tile([128, 24], FP32, tag="dcol")
        tmp = sb_win.tile([128, CZ], FP32, tag="f1")
        nc.vector.tensor_mul(tmp[:], qf[:], qx[:, 256:512])
        nc.vector.tensor_reduce(dcol[:, 0:8], tmp[:].rearrange("p (h c) -> p h c", c=CO), AX.X, ALU.add)
        nc.vector.tensor_mul(tmp[:], qf[:], cs["skb"][:])
        nc.vector.tensor_reduce(dcol[:, 8:16], tmp[:].rearrange("p (h c) -> p h c", c=CO), AX.X, ALU.add)
        nc.vector.tensor_mul(tmp[:], qf[:], cs["ckb"][:])
        nc.vector.tensor_reduce(dcol[:, 16:24], tmp[:].rearrange("p (h c) -> p h c", c=CO), AX.X, ALU.add)
        G = sb_win.tile([128, 280], BF16, tag="G")
        nc.scalar.copy(G[:, 0:256], qf[:])
        nc.vector.tensor_copy(G[:, 256:280], dcol[:, 0:24])

        xcv_ps = p_kv.tile([128, 512], FP32, tag="kvp")
        for k in range(2):
            nc.tensor.matmul(xcv_ps[:, 0:256], xwt[:, k, :],
                             cs["cv"][:, k, :],
                             start=(k == 0), stop=(k == 1))
        xcv = sb_win.tile([128, CZ], FP32, tag="xcv")
        nc.scalar.copy(xcv[:], xcv_ps[:, 0:256])

        # ea for the whole window (bf16, HWDGE)
        eaw = sb_ea.tile([CE, W_E], BF16, tag="eaw")
        nc.sync.dma_start(eaw[:], ea_t[:, w * W_E:(w + 1) * W_E])

        if stage < 3:
            nc.sync.dma_start(y_d[w * 128:(w + 1) * 128, :], xcv[:])
            continue

        scat = p_scat.tile([128, 280], FP32, tag="scat")

        # ---- window-level onehots, transposes, q-matmuls ----
        OHew = sb_oh.tile([128, JPW, 128], BF16, tag="OHew")
        OHdw = sb_od.tile([128, JPW, 128], BF16, tag="OHdw")
        q_sbw = sb_oh.tile([128, JPW, 280], BF16, tag="q_sbw")
        # drel replicated across partitions for this window (bf16)
        drw = sb_dr.tile([128, W_E], BF16, tag="drw")
        nc.sync.dma_start(drw[:], drelr_d[:, w * W_E:(w + 1) * W_E])
        # OHe[e, jj, d] = (drel[e, jj] == d); OHd[d, jj, e] = (drelr[jj, e] == p)
        nc.vector.tensor_tensor(
            OHew[:], drel_sb[:, w * JPW:(w + 1) * JPW].unsqueeze(2).broadcast_to([128, JPW, 128]),
            cs["iota"][:].unsqueeze(1).broadcast_to([128, JPW, 128]), ALU.is_equal)
        nc.vector.tensor_scalar(
            OHdw[:], drw[:].rearrange("p (j e) -> p j e", e=128), cs["iotap"][:], None,
            ALU.is_equal)
        for jj in range(JPW):
            qp = p_q.tile([128, 280], FP32, tag="qp")
            nc.tensor.matmul(qp[:], OHdw[:, jj, :], G[:])
            nc.scalar.copy(q_sbw[:, jj, :], qp[:])

        # ---- edge superblocks ----
        for g in range(NSB):
            gsb = w * NSB + g
            GT = sb_gt.tile([128, JPS, TROW], BF16, tag="GT")
            nc.gpsimd.dma_gather(GT[:], tsrc, idx_sb[:, gsb * IPS:(gsb + 1) * IPS],
                                 SBE, SBE, TROW)
            ea_sb = sb_work.tile([128, JPS, 512], BF16, tag="ea_sb")
            for j in range(JPS):
                kvp = p_kv.tile([128, 512], FP32, tag="kvp")
                ej = eaw[:, (g * JPS + j) * 128:(g * JPS + j + 1) * 128]
                nc.tensor.matmul(kvp[:], ej, cs["akv"][:])
                nc.scalar.copy(ea_sb[:, j, :], kvp[:])

            # ---- DVE math on full superblock ----
            q_sb = q_sbw[:, g * JPS:(g + 1) * JPS, :]
            kvs = sb_msg.tile([128, JPS, 256], BF16, tag="kvs")
            kvsv = sb_kvv.tile([128, JPS, 256], BF16, tag="kvsv")
            lc = sb_msg.tile([128, JPS, 8], BF16, tag="lc")
            lg = sb_msg.tile([128, JPS, 8], FP32, tag="lg")
            lg2 = sb_msg.tile([128, JPS, 8], FP32, tag="lg2")
            msg = sb_msg.tile([128, JPS, 280], BF16, tag="msg")
            prod = msg[:, :, 0:256]  # scratch; overwritten by value message later

            c0 = w * JPW + g * JPS
            inv_b = inv_sb[:, c0:c0 + JPS].unsqueeze(2).broadcast_to([128, JPS, 8])
            muinv_b = muinv_sb[:, c0:c0 + JPS].unsqueeze(2).broadcast_to([128, JPS, 8])

            nc.vector.tensor_add(kvs[:], GT[:, :, 0:256], ea_sb[:, :, 0:256])
            nc.vector.tensor_mul(prod, q_sb[:, :, 0:256], kvs[:])
            with nc.allow_low_precision("bf16 logit partial sums"):
                nc.vector.tensor_reduce(lc[:], prod.rearrange("p s (h c) -> p s h c", c=CO),
                                        AX.X, ALU.add)
            # logits = inv*(lc + d1g) - (mu*inv)*d2g + d3g
            nc.vector.tensor_add(lg[:], lc[:], q_sb[:, :, 256:264])
            nc.vector.tensor_mul(lg[:], lg[:], inv_b)
            nc.vector.tensor_mul(lg2[:], q_sb[:, :, 264:272], muinv_b)
            nc.vector.tensor_sub(lg[:], lg[:], lg2[:])
            nc.vector.tensor_add(lg[:], lg[:], q_sb[:, :, 272:280])
            # ex, u1, u2 -> msg[:, :, 256:280]
            nc.scalar.activation(msg[:, :, 256:264], lg[:], AF.Exp)
            nc.vector.tensor_mul(msg[:, :, 264:272], msg[:, :, 256:264], inv_b)
            nc.vector.tensor_mul(msg[:, :, 272:280], msg[:, :, 256:264], muinv_b)
            # value message (v-sum is independent of the logit chain)
            nc.vector.tensor_add(kvsv[:], GT[:, :, 256:512], ea_sb[:, :, 256:512])
            u1_b = msg[:, :, 264:272].unsqueeze(3).broadcast_to([128, JPS, 8, CO])
            nc.vector.tensor_mul(msg[:, :, 0:256], u1_b,
                                 kvsv[:].rearrange("p s (h c) -> p s h c", c=CO))
            # scatter
            for j in range(JPS):
                nc.tensor.matmul(scat[:], OHew[:, g * JPS + j, :], msg[:, j, :],
                                 start=(g == 0 and j == 0),
                                 stop=(g == NSB - 1 and j == JPS - 1),
                                 skip_group_check=True)

        if stage < 4:
            dbg2 = sb_win.tile([128, CZ], FP32, tag="dbg2")
            nc.vector.tensor_copy(dbg2[:], scat[:, 0:256])
            nc.sync.dma_start(y_d[w * 128:(w + 1) * 128, :], dbg2[:])
            continue

        # ---- window finalize ----
        att = sb_win.tile([128, CZ], FP32, tag="att")
        f1 = sb_win.tile([128, CZ], FP32, tag="f1")
        recD = sb_win.tile([128, 16], FP32, tag="recD")
        nc.vector.tensor_scalar_max(recD[:, 8:16], scat[:, 256:264], 1e-30)
        nc.vector.reciprocal(recD[:, 0:8], recD[:, 8:16])
        u1w = scat[:, 264:272].unsqueeze(2).broadcast_to([128, 8, CO])
        u2w = scat[:, 272:280].unsqueeze(2).broadcast_to([128, 8, CO])
        rD = recD[:, 0:8].unsqueeze(2).broadcast_to([128, 8, CO])
        nc.vector.tensor_mul(f1[:].rearrange("p (h c) -> p h c", c=CO),
                             xcv[:].rearrange("p (h c) -> p h c", c=CO), u1w)
        nc.vector.tensor_add(att[:], scat[:, 0:256], f1[:])
        nc.vector.tensor_mul(f1[:].rearrange("p (h c) -> p h c", c=CO),
                             cs["svb"][:].rearrange("p (h c) -> p h c", c=CO), u2w)
        nc.vector.tensor_sub(att[:], att[:], f1[:])
        nc.vector.tensor_mul(att[:].rearrange("p (h c) -> p h c", c=CO),
                             att[:].rearrange("p (h c) -> p h c", c=CO), rD)
        nc.vector.tensor_add(att[:], att[:], cs["cvb"][:])
        if stage < 5:
            nc.sync.dma_start(y_d[w * 128:(w + 1) * 128, :], att[:])
            continue

        # ---- MLP ----
        attb = sb_win.tile([128, CZ], BF16, tag="attb")
        nc.vector.tensor_copy(attb[:], att[:])
        tp2 = p_tpb.tile([128, 4, 128], BF16, tag="tpb")
        at_t = sb_win.tile([128, 2, 128], BF16, tag="at_t")
        for k in range(2):
            transpose_128(tp2[:, k, :], attb[:, k * 128:(k + 1) * 128])
        nc.scalar.copy(at_t[:], tp2[:, 0:2, :])
        h1 = p_kv.tile([128, 512], FP32, tag="kvp")
        for k in range(2):
            nc.tensor.matmul(h1[:], at_t[:, k, :],
                             cs["w1"][:, k, :],
                             start=(k == 0), stop=(k == 1))
        hs = sb_win.tile([128, 512], FP32, tag="hs")
        sg = sb_win.tile([128, 512], BF16, tag="sg")
        nc.vector.tensor_add(hs[:], h1[:], cs["b1b"][:])
        nc.scalar.activation(sg[:], hs[:], AF.Sigmoid)
        hsb = sb_win.tile([128, 512], BF16, tag="hsb")
        nc.vector.tensor_mul(hsb[:], hs[:], sg[:])
        tp3 = p_tpb.tile([128, 4, 128], BF16, tag="tpb")
        h_t = sb_win.tile([128, 4, 128], BF16, tag="h_t")
        for k in range(4):
            transpose_128(tp3[:, k, :], hsb[:, k * 128:(k + 1) * 128])
        nc.scalar.copy(h_t[:], tp3[:])
        yp = p_kv.tile([128, 512], FP32, tag="kvp")
        for k in range(4):
            nc.tensor.matmul(yp[:, 0:256], h_t[:, k, :],
                             cs["w2"][:, k, :],
                             start=(k == 0), stop=(k == 3))
        ys = sb_win.tile([128, CZ], FP32, tag="ys")
        nc.vector.tensor_add(ys[:], yp[:, 0:256], cs["b2b"][:])
        nc.sync.dma_start(y_d[w * 128:(w + 1) * 128, :], ys[:])


_CACHE = {}


def kernel_ex(**inputs):
    key = "k"
    cfg = Cfg(NCORES=NCORES, NLOC=NLOC, NLOC_PAD=NLOC_PAD, NWIN=NWIN,
              NPAD=NPAD, NTB=NTB)
    consts_h, in_maps, W_E, NSB = _host_prep(cfg, **inputs)
    cfg.W_E, cfg.NSB = W_E, NSB
    if key not in _CACHE:
        nc = bacc.Bacc("TRN2", target_bir_lowering=False, debug=False,
                       num_devices=NCORES)
        with tile.TileContext(nc, trace_sim=False) as tc:
            with ExitStack() as ctx:
                _build(nc, tc, ctx, consts_h, cfg)
        nc.compile()
        _CACHE[key] = nc
    nc = _CACHE[key]
    res = bass_utils.run_bass_kernel_spmd(nc, in_maps, core_ids=list(range(NCORES)))
    out = np.zeros((N, CZ), np.float32)
    for c in range(NCORES):
        out[c * NLOC:(c + 1) * NLOC] = res.results[c]["y"][:NLOC]
    return out, res


def kernel(**inputs):
    return kernel_ex(**inputs)[0]
